# revision 2
# baseline (speedup 1.0000x reference)
"""Trainium2 Bass kernel for the ragged text-CNN problem (v3).

Math: for tokens x[t,b] with embedding tables E,U [V,D] and conv
weights w [H, 2D, 2] (kernel size 2 over time):

    conv[b,h,t] = w0_h . e_{t,b} + w1_h . e_{t+1,b} + cb_h
    scores      = (max over valid t of conv) @ out_w.T + out_b

where e = concat(E[tok], U[tok]).  We precompute a fused table
T[v, 0:64] = e_v . w0, T[v, 64:128] = e_v . w1, so
conv[b,h,t] = T[tok_t, h] + T[tok_{t+1}, 64+h].  The ragged mask is
free: PAD (=1) rows get T[1, 0:64] = -1e30, which masks every conv
position t >= len.

Distribution/layout:
 - Vocab padded to 51200 rows, sharded 6400 rows/core.  The host ships
   X^T = concat(E,U)^T in bf16; the device builds its table shard with
   a tiled matmul against the host-packed conv weights.  Output tiles
   land in natural row layout: row v at partition v%128, 256B SBUF
   stripe v//128.
 - Shards are exchanged SBUF->SBUF with one remote_dma_broadcast per
   core (8 dests incl. self-loopback, dynamic slice offset = core id).
   Receivers gate phase B on the remote semaphore; its threshold is
   loaded from an input tensor into a register so the tile scheduling
   pass (no-exec; reg reads 0) does not deadlock on a semaphore that
   only remote cores increment.
 - Phase B: two SBUF-source dma_gathers (transpose mode) fetch each
   token's 256B table row: int16 gather indices cannot span 51200
   rows, so the low gather covers rows [0, 25600) and the high gather
   rows [25600, 51200), with out-of-half positions pointed at rows
   patched to -1e30 (row 0 / row 51199); an elementwise max merges the
   halves.  conv = A-half + shifted B-half; the max-pool view skips
   t=511 (the cross-sentence column) and positions >= len are masked
   by the PAD patch.  A [65, 32] x [65, 2] matmul applies the head.
"""

import numpy as np

try:
    import concourse.bass as bass
except ImportError:  # harness runs from a bare directory
    import sys

    sys.path.insert(0, "/opt/trn_rl_repo")
    import concourse.bass as bass

import ml_dtypes
import concourse.mybir as mybir
from concourse.bacc import Bacc
import concourse.tile as tile
from concourse.bass_utils import run_bass_kernel_spmd

# ---------------------------------------------------------------------------
# The simulator resolves RDMA core routing via the neuron driver's
# logical->physical NC maps; fall back to identity maps when no driver is
# present (pure-simulation environments).  Real-hardware runs never hit the
# fallback because the driver ioctls succeed there.
import concourse.libnrt as libnrt

_orig_nc_map = libnrt.get_trn2_nc_mapping


def _nc_map_with_fallback():
    try:
        return _orig_nc_map()
    except Exception:
        return {(d, k): k for d in range(16) for k in range(8)}


libnrt.get_trn2_nc_mapping = _nc_map_with_fallback

_orig_rid_map = libnrt.get_device_id_to_routing_id_mapping


def _rid_map_with_fallback():
    try:
        return _orig_rid_map()
    except Exception:
        return {d: d for d in range(16)}


libnrt.get_device_id_to_routing_id_mapping = _rid_map_with_fallback

import concourse.bass_interp as _bass_interp

_bass_interp.get_device_id_to_routing_id_mapping = _rid_map_with_fallback
# ---------------------------------------------------------------------------

V, D, H, S, B = 50000, 300, 64, 512, 256
NCORES = 8
VP = 51200              # padded vocab (8 * 6400)
VS = VP // NCORES       # vocab rows per core (6400)
VH = VP // 2            # gather half size (25600)
BS = B // NCORES        # sentences per core (32)
F = 2 * H               # fused feature width (128)
NEG = -1.0e30
P = 128
NT = VS // P            # table tiles per core (50)
KCH = [128, 128, 128, 128, 88]   # contraction chunks over 2D=600
NBLK = 5                # xt column blocks (1280 cols each)
BLKW = VS // NBLK       # 1280
NTOK = S * BS           # tokens per core (16384)
CH = 2048               # merge/conv chunk (4 sentences)

F32 = mybir.dt.float32
BF16 = mybir.dt.bfloat16
I16 = mybir.dt.int16
I32 = mybir.dt.int32


def build_nc():
    nc = Bacc(dynamic_dma_scratch_size=16448)
    xt = nc.dram_tensor("xt", [2 * D, VS], BF16, kind="ExternalInput")
    w2 = nc.dram_tensor("w2", [P, 5 * P], BF16, kind="ExternalInput")
    idx = nc.dram_tensor("idx", [P, 2 * (NTOK // 16)], I16, kind="ExternalInput")
    patcha = nc.dram_tensor("patcha", [2, F], F32, kind="ExternalInput")
    patchb = nc.dram_tensor("patchb", [1, F], F32, kind="ExternalInput")
    cb = nc.dram_tensor("cb", [H, 1], F32, kind="ExternalInput")
    ow = nc.dram_tensor("ow", [H + 1, 2], BF16, kind="ExternalInput")
    thr = nc.dram_tensor("thr", [1, 1], I32, kind="ExternalInput")
    scores = nc.dram_tensor("scores", [BS, 2], F32, kind="ExternalOutput")

    lsem = nc.alloc_semaphore("tbl_lsem")
    rsem = nc.alloc_semaphore("tbl_rsem")
    adh = bass._add_dep_helper

    with tile.TileContext(nc) as tc:
        with tc.tile_pool(name="const", bufs=1) as cpool:
            # persistent full table, row layout: row v at partition v%128,
            # elems [128*(v//128), +128)
            tbl = cpool.tile([P, VP], BF16, tag="tbl")
            w2_sb = cpool.tile([P, 5 * P], BF16, tag="w2")
            nc.sync.dma_start(w2_sb[:, :], w2[:, :])
            pa_sb = cpool.tile([2, F], F32, tag="patcha")
            nc.sync.dma_start(pa_sb[:, :], patcha[:, :])
            pb_sb = cpool.tile([1, F], F32, tag="patchb")
            nc.sync.dma_start(pb_sb[:, :], patchb[:, :])
            idx_sb = cpool.tile([P, 2 * (NTOK // 16)], I16, tag="idx")
            nc.sync.dma_start(idx_sb[:, :], idx[:, :])
            thr_sb = cpool.tile([1, 1], I32, tag="thr")
            nc.gpsimd.dma_start(thr_sb[:, :], thr[:, :])
            shard = cpool.tile([P, VS], BF16, tag="shard")

            # ---- Phase A: table shard build -------------------------------
            # DMA "processing" is charged to the issuing engine, so spread
            # the X^T block loads across three engines.
            load_eng = [nc.sync, nc.sync, nc.scalar, nc.scalar, nc.gpsimd]
            with (
                tc.tile_pool(name="pa", bufs=2) as xpool,
                tc.tile_pool(name="pa_ps", bufs=6, space="PSUM") as pspool,
            ):
                for blk in range(NBLK):
                    xts = []
                    for ci, kc in enumerate(KCH):
                        xtile = xpool.tile([P, BLKW], BF16, tag=f"x{ci}")
                        load_eng[ci].dma_start(
                            xtile[:kc, :],
                            xt[ci * P : ci * P + kc,
                               blk * BLKW : (blk + 1) * BLKW],
                        )
                        xts.append(xtile)
                    for jj in range(BLKW // P):
                        t = blk * (BLKW // P) + jj
                        ps = pspool.tile([P, P], F32, tag="ps")
                        for ci, kc in enumerate(KCH):
                            nc.tensor.matmul(
                                ps[:, :],
                                lhsT=xts[ci][:kc, jj * P : (jj + 1) * P],
                                rhs=w2_sb[:kc, ci * P : (ci + 1) * P],
                                start=(ci == 0),
                                stop=(ci == len(KCH) - 1),
                            )
                        if t == 0:
                            # rows 0..1: low-half dummy row and the PAD
                            # A-half mask (-1e30 on core 0, zero elsewhere)
                            nc.vector.tensor_tensor(
                                ps[0:2, :], ps[0:2, :], pa_sb[:, :],
                                op=mybir.AluOpType.add,
                            )
                        if t == NT - 1:
                            # row 51199: high-half dummy row (core 7)
                            nc.vector.tensor_tensor(
                                ps[P - 1 : P, :], ps[P - 1 : P, :],
                                pb_sb[:, :], op=mybir.AluOpType.add,
                            )
                        nc.vector.tensor_copy(
                            shard[:, t * P : (t + 1) * P], ps[:, :]
                        )

            # ---- shard exchange: SBUF broadcast to all 8 cores ------------
            # One broadcast per 10-tile group so descriptor generation
            # overlaps the remaining phase-A compute; only the last group's
            # desc-gen sits on the critical path.
            pid = nc.gpsimd.partition_id()
            NBC = NBLK
            bw_ = VS // NBC
            for g in range(NBC):
                nc.gpsimd.remote_dma_broadcast(
                    tbl[:, bass.DynSlice(pid * VS + g * bw_, bw_)],
                    shard[:, g * bw_ : (g + 1) * bw_],
                    remote_sem=rsem,
                    local_sem=lsem,
                    rdests=[(0, k) for k in range(NCORES)],
                )
                tr = nc.gpsimd.trigger_dma(count=None)
            # rsem reaches 16*NBC when every sender's data (incl. our own
            # loopback) has landed.
            treg = nc.gpsimd.alloc_register("rsem_thr")
            rl = nc.gpsimd.reg_load(treg, thr_sb[0:1, 0:1])
            tval = nc.gpsimd.snap(treg, donate=True, min_val=0, max_val=255)
            w = nc.gpsimd.wait_ge(rsem, tval)
            adh(w.ins, tr.ins, sync=True, reason="rsem wait after trigger")
            adh(w.ins, rl.ins, sync=False, reason="rsem wait after reg load")
            # also wait for our own send completion so the SWDGE ring can
            # reclaim the broadcast descriptor entries before the gathers
            treg2 = nc.gpsimd.alloc_register("lsem_thr")
            rl2 = nc.gpsimd.reg_load(treg2, thr_sb[0:1, 0:1])
            tval2 = nc.gpsimd.snap(treg2, donate=True, min_val=0, max_val=255)
            wl = nc.gpsimd.wait_ge(lsem, tval2)
            adh(wl.ins, tr.ins, sync=True, reason="lsem wait after trigger")
            adh(wl.ins, rl2.ins, sync=False, reason="lsem wait after reg load")
            adh(wl.ins, w.ins, sync=True, reason="order sem waits")

            # ---- Phase B: gathers + merge + conv + pool + head ------------
            with (
                tc.tile_pool(name="pb", bufs=1) as pbpool,
                tc.tile_pool(name="pbc", bufs=2) as spool,
                tc.tile_pool(name="pb_ps", bufs=2, space="PSUM") as pbpsum,
            ):
                pooledX = pbpool.tile([H + 1, BS], BF16, tag="pooledX")
                ms = nc.vector.memset(pooledX[H : H + 1, :], 1.0)
                cb_sb = pbpool.tile([H, 1], F32, tag="cb")
                d1 = nc.sync.dma_start(cb_sb[:, :], cb[:, :])
                ow_sb = pbpool.tile([H + 1, 2], BF16, tag="ow")
                d2 = nc.sync.dma_start(ow_sb[:, :], ow[:, :])
                # these tiles may land in the freed shard region, whose
                # deferred RDMA read is only signalled via lsem (wl)
                for _i in (ms, d1, d2):
                    adh(_i.ins, wl.ins, sync=True,
                        reason="phase-B alloc after shard fully sent")

                HW2 = NTOK // 2
                glo = pbpool.tile([P, NTOK], BF16, tag="g0")
                with tc.tile_pool(name="pbg", bufs=1) as gpool:
                    ghi = gpool.tile([P, NTOK], BF16, tag="g1")
                    for hf, g in ((0, glo), (1, ghi)):
                        gi = nc.gpsimd.dma_gather(
                            out_ap=g[:].rearrange("p (l i) -> p l i", l=1),
                            in_ap=tbl[:, hf * VH : (hf + 1) * VH],
                            idxs_ap=idx_sb[
                                :, hf * (NTOK // 16) : (hf + 1) * (NTOK // 16)
                            ],
                            num_idxs=NTOK,
                            num_idxs_reg=NTOK,
                            elem_size=F,
                            transpose=True,
                            queue_num=0,
                            single_packet=False,
                            sbuf_tokens_per_rank=P,
                            sbuf_free_dim_per_rank=256,
                            sbuf_free_dim_pad_per_rank=0,
                            sbuf_byte_offset=0,
                        )
                        adh(gi.ins, w.ins, sync=True,
                            reason="gather after remote shards landed")
                        adh(gi.ins, wl.ins, sync=True,
                            reason="gather after ring reclaimable")
                    # merge halves (dummy rows are -1e30), in place into glo;
                    # position-half 0 on DVE, half 1 on Pool, concurrently
                    nc.vector.tensor_tensor(
                        glo[:, 0:HW2], glo[:, 0:HW2], ghi[:, 0:HW2],
                        op=mybir.AluOpType.max,
                    )
                    nc.gpsimd.tensor_tensor(
                        glo[:, HW2:NTOK], glo[:, HW2:NTOK], ghi[:, HW2:NTOK],
                        op=mybir.AluOpType.max,
                    )
                # conv + ragged max-pool + head, one position half (16
                # sentences) at a time; conv tiles reuse ghi's SBUF space.
                # Each half is self-contained: its last column (t=511 of its
                # last sentence) is the cross-sentence column, which the
                # pooling view skips, so conv[i] never reads m[i+1] across
                # the half boundary.
                with (
                    tc.tile_pool(name="pbv", bufs=1) as vpool,
                ):
                    for hf, eng in ((0, nc.vector), (1, nc.gpsimd)):
                        i0 = hf * HW2
                        sl = slice(hf * (BS // 2), (hf + 1) * (BS // 2))
                        conv = vpool.tile([H, HW2], BF16, tag=f"conv{hf}")
                        eng.tensor_tensor(
                            conv[:, 0 : HW2 - 1],
                            glo[0:H, i0 : i0 + HW2 - 1],
                            glo[H:P, i0 + 1 : i0 + HW2],
                            op=mybir.AluOpType.add,
                        )
                        # max over t = 0..510 per sentence (t=511 is the
                        # cross-sentence column; positions >= len hold the
                        # -1e30 PAD patch in their A-half)
                        nc.vector.reduce_max(
                            pooledX[0:H, sl],
                            conv[:].rearrange(
                                "p (b t) -> p b t", t=S
                            )[:, :, 0 : S - 1],
                            axis=mybir.AxisListType.X,
                        )
                        nc.vector.tensor_scalar_add(
                            pooledX[0:H, sl], pooledX[0:H, sl], cb_sb[:, :]
                        )
                        sc_ps = pbpsum.tile([BS // 2, 2], F32, tag="sc")
                        nc.tensor.matmul(
                            sc_ps[:, :],
                            lhsT=pooledX[:, sl],
                            rhs=ow_sb[:, :],
                            start=True,
                            stop=True,
                        )
                        sc_sb = pbpool.tile(
                            [BS // 2, 2], F32, tag=f"sc_sb{hf}"
                        )
                        eng.tensor_copy(sc_sb[:, :], sc_ps[:, :])
                        nc.sync.dma_start(scores[sl, :], sc_sb[:, :])

    nc.finalize()
    return nc


_NC_CACHE = {}


def _get_nc():
    if "nc" not in _NC_CACHE:
        _NC_CACHE["nc"] = build_nc()
    return _NC_CACHE["nc"]


def make_in_maps(sentences, E, U, conv_w, conv_b, out_w, out_b):
    bf16 = ml_dtypes.bfloat16
    Xp = np.zeros((VP, 2 * D), np.float32)
    Xp[:V, :D] = E
    Xp[:V, D:] = U
    # W2[c, k*64+h] = conv_w[h, c, k], packed into 128-row chunks
    W2 = np.ascontiguousarray(
        conv_w.transpose(1, 2, 0).reshape(2 * D, F)
    ).astype(np.float32)
    w2p = np.zeros((P, 5 * P), np.float32)
    for ci, kc in enumerate(KCH):
        w2p[:kc, ci * P : (ci + 1) * P] = W2[ci * P : ci * P + kc, :]
    w2p = w2p.astype(bf16)

    ow = np.concatenate(
        [out_w.T.astype(np.float32), out_b.reshape(1, 2).astype(np.float32)], 0
    ).astype(bf16)
    cbv = conv_b.reshape(H, 1).astype(np.float32)

    def wrap(ix):
        return np.ascontiguousarray(ix.reshape(NTOK // 16, 16).T)

    in_maps = []
    for c in range(NCORES):
        xtc = np.ascontiguousarray(
            Xp[c * VS : (c + 1) * VS].T
        ).astype(bf16)
        toks = np.ascontiguousarray(
            sentences[:, c * BS : (c + 1) * BS].T
        ).reshape(-1).astype(np.int64)
        lo = toks < VH
        idx_lo = np.where(lo, toks, 0).astype(np.int16)
        idx_hi = np.where(lo, VH - 1, toks - VH).astype(np.int16)
        idx16 = np.tile(
            np.concatenate([wrap(idx_lo), wrap(idx_hi)], axis=1), (8, 1)
        )
        pa = np.zeros((2, F), np.float32)
        pb = np.zeros((1, F), np.float32)
        if c == 0:
            pa[0, :] = NEG          # row 0: low-half dummy target
            pa[1, :H] = NEG         # row 1: PAD token A-half mask
        if c == NCORES - 1:
            pb[0, :] = NEG          # row 51199: high-half dummy target
        in_maps.append(
            {
                "xt": xtc,
                "w2": w2p,
                "idx": idx16,
                "patcha": pa,
                "patchb": pb,
                "cb": cbv,
                "ow": ow.copy(),
                "thr": np.full((1, 1), 80, np.int32),
            }
        )
    return in_maps


def kernel(sentences, E, U, conv_w, conv_b, out_w, out_b):
    sentences = np.asarray(sentences, dtype=np.int32)
    E = np.asarray(E, dtype=np.float32)
    U = np.asarray(U, dtype=np.float32)
    conv_w = np.asarray(conv_w, dtype=np.float32)
    conv_b = np.asarray(conv_b, dtype=np.float32)
    out_w = np.asarray(out_w, dtype=np.float32)
    out_b = np.asarray(out_b, dtype=np.float32)

    nc = _get_nc()
    in_maps = make_in_maps(sentences, E, U, conv_w, conv_b, out_w, out_b)
    res = run_bass_kernel_spmd(nc, in_maps, list(range(NCORES)))
    return np.concatenate(
        [res.results[c]["scores"] for c in range(NCORES)], axis=0
    )
